# revision 3
# baseline (speedup 1.0000x reference)
"""Trainium2 Bass kernel for nn_CAC_42511586296007 (circular-mask max-pool descriptor).

Reference computation (per batch b, channel c):
  v = l2norm_over_c(max_hw(x)) + sum over 153 circular masks m of
      l2norm_over_c(max_hw(x * m))
Masks are center point + per-quadrant rings/circles of integer radius.

Decomposition used here: every mask max is derived from 77 "segment" maxes
(center + ring(q, r) for q in 0..3, r in 1..19) plus an 'outer corners'
segment (cells with r2 > 361) that only feeds the full-map max:
  circle(q, r) = max(center, ring(q, 1..r))   (prefix max)
  masked outputs are clamped at 0 (masks always contain zeros)
  full max = max over all 78 segments
Verified bit-exact against the reference mask construction in numpy.

Data-parallel over batch: 8 cores x 4 batches. Per core:
  phase 1: per (b, ctile) tile [128c x 784s]: DMA load, GPSIMD ap_gather into
           ring-segment-sorted padded layout, grouped DVE segmented reduce_max.
  phase 2: full-max, relu clamp, second small gather into per-quadrant prefix
           layout, serial prefix max, square / channel-norms via PE matmuls,
           1/(norm+eps), broadcast back, multiply-accumulate over 154 slots.
"""

import numpy as np

_B, _C, _HH, _WW = 32, 1024, 28, 28
_S = _HH * _WW            # 784
_NCORES = 8
_BL = _B // _NCORES       # 4 batches per core
_CT = _C // 128           # 8 channel tiles
_NT = _BL * _CT           # 32 tiles per core
_MAXR = 20
_NSLOT = 154              # full + center + 76 rings + 76 circles
_EPS = 1e-6


# --------------------------------------------------------------------------
# constant tables (host, compile-time)
# --------------------------------------------------------------------------

def _segments():
    ci = np.arange(_HH).reshape(-1, 1) - _HH // 2
    cj = np.arange(_WW).reshape(1, -1) - _WW // 2
    r2 = ci * ci + cj * cj
    ring_id = np.ceil(np.sqrt(r2)).astype(int)
    quadrants = [(1, 1), (-1, 1), (1, -1), (-1, -1)]
    segs = [np.flatnonzero((r2 == 0).ravel()).tolist()]
    for sx, sy in quadrants:
        quad = (sx * cj >= 0) & (sy * ci >= 0)
        for r in range(1, _MAXR):
            segs.append(np.flatnonzero(((ring_id == r) & quad).ravel()).tolist())
    segs.append(np.flatnonzero((ring_id >= _MAXR).ravel()).tolist())
    return segs


def _optimal_classes(sizes, overhead=58):
    order = np.argsort(sizes, kind="stable")
    ss = [sizes[i] for i in order]
    n = len(ss)
    INF = 1 << 30
    dp = [0] + [INF] * n
    choice = [0] * (n + 1)
    for i in range(1, n + 1):
        for j in range(i):
            cost = dp[j] + overhead + sum(ss[i - 1] - ss[k] for k in range(j, i))
            if cost < dp[i]:
                dp[i] = cost
                choice[i] = j
    bounds = []
    i = n
    while i > 0:
        bounds.append((choice[i], i))
        i = choice[i]
    bounds.reverse()
    return order, ss, bounds


def _build_tables():
    segs = _segments()                      # 78 segment cell lists
    sizes = [len(c) for c in segs[:77]]
    order, ss, bounds = _optimal_classes(sizes)

    classes = []
    for j, i in bounds:
        classes.append((ss[i - 1], [int(order[k]) for k in range(j, i)]))
    outer_pad = -(-len(segs[77]) // 4) * 4
    classes.append((outer_pad, [77]))

    classorder = [s for _, members in classes for s in members]
    pos_of_sem = {s: p for p, s in enumerate(classorder)}

    idx1 = []
    class_meta = []                         # (elem_offset, n_segs, S, segpos)
    segpos = 0
    for S, members in classes:
        class_meta.append((len(idx1), len(members), S, segpos))
        for s in members:
            cells = segs[s]
            idx1.extend(cells + [cells[0]] * (S - len(cells)))
        segpos += len(members)
    npad0 = len(idx1)
    NPAD = -(-npad0 // 16) * 16
    idx1.extend([0] * (NPAD - npad0))
    idx1 = np.asarray(idx1, dtype=np.int16)

    def wrap(idx, num_idxs):
        w = idx.reshape(num_idxs // 16, 16).T
        return np.ascontiguousarray(np.tile(w, (8, 1)))

    idx2 = np.zeros(_NT * 80, dtype=np.int16)
    p_center = pos_of_sem[0]
    for i in range(_NT):
        base = i * _NSLOT
        for q in range(4):
            idx2[i * 80 + q * 20 + 0] = base + 1 + p_center
            for r in range(1, 20):
                sem = 1 + q * 19 + (r - 1)
                idx2[i * 80 + q * 20 + r] = base + 1 + pos_of_sem[sem]

    return dict(
        class_meta=class_meta, NPAD=NPAD,
        idx1_w=wrap(idx1, NPAD), idx2_w=wrap(idx2, _NT * 80),
    )


_TABLES = _build_tables()


# --------------------------------------------------------------------------
# bass kernel build
# --------------------------------------------------------------------------

_NC_CACHE = None


def _build_nc():
    import concourse.bacc as bacc
    import concourse.mybir as mybir
    from concourse.tile import TileContext

    t = _TABLES
    NPAD = t["NPAD"]
    class_meta = t["class_meta"]
    f32 = mybir.dt.float32
    i16 = mybir.dt.int16
    AX = mybir.AxisListType
    AF = mybir.ActivationFunctionType
    GRP = 8                                # tiles per reduce group
    NGRP = _NT // GRP

    nc = bacc.Bacc("TRN2")
    xs = nc.dram_tensor("xs", [_BL, _C, _S], f32, kind="ExternalInput")
    idx1_d = nc.dram_tensor("idx1", [128, NPAD // 16], i16, kind="ExternalInput")
    idx2_d = nc.dram_tensor("idx2", [128, (_NT * 80) // 16], i16, kind="ExternalInput")
    ones128_d = nc.dram_tensor("ones128", [128, 1], f32, kind="ExternalInput")
    ones1_d = nc.dram_tensor("ones1", [1, 128], f32, kind="ExternalInput")
    out_d = nc.dram_tensor("out", [128, _NT], f32, kind="ExternalOutput")

    with TileContext(nc) as tc:
        with (
            tc.tile_pool(name="const", bufs=1) as cpool,
            tc.tile_pool(name="x", bufs=6) as xpool,
            tc.tile_pool(name="g", bufs=2) as gpool,
            tc.tile_pool(name="big", bufs=1) as bpool,
            tc.tile_pool(name="small", bufs=1) as spool,
            tc.tile_pool(name="psn", bufs=4, space="PSUM") as ppool_n,
            tc.tile_pool(name="pbc", bufs=2, space="PSUM") as ppool_b,
        ):
            idx1_t = cpool.tile([128, NPAD // 16], i16, tag="idx1")
            nc.sync.dma_start(out=idx1_t[:], in_=idx1_d[:])
            idx2_t = cpool.tile([128, (_NT * 80) // 16], i16, tag="idx2")
            nc.sync.dma_start(out=idx2_t[:], in_=idx2_d[:])
            ones128_t = cpool.tile([128, 1], f32, tag="ones128")
            nc.sync.dma_start(out=ones128_t[:], in_=ones128_d[:])
            ones1_t = cpool.tile([1, 128], f32, tag="ones1")
            nc.sync.dma_start(out=ones1_t[:], in_=ones1_d[:])

            seg = bpool.tile([128, _NT * 78], f32, tag="seg")
            seg_r = seg[:].rearrange("p (i k) -> p i k", i=_NT)
            vt = bpool.tile([128, _NT * _NSLOT], f32, tag="vt")
            vt_r = vt[:].rearrange("p (i k) -> p i k", i=_NT)

            # ---- phase 1: load / gather / segmented reduce ----
            for grp in range(NGRP):
                gt = gpool.tile([128, GRP * NPAD], f32, tag="gt")
                for li in range(GRP):
                    i = grp * GRP + li
                    b, ct = divmod(i, _CT)
                    xt = xpool.tile([128, _S], f32, tag="xt")
                    nc.sync.dma_start(
                        out=xt[:], in_=xs[b, ct * 128:(ct + 1) * 128, :]
                    )
                    nc.gpsimd.ap_gather(
                        out_ap=gt[:, li * NPAD:(li + 1) * NPAD],
                        in_ap=xt[:],
                        idxs_ap=idx1_t[:],
                        channels=128, num_elems=_S, d=1, num_idxs=NPAD,
                    )
                gt_r = gt[:].rearrange("p (l x) -> p l x", l=GRP)
                for off, n, S, sp in class_meta:
                    in_ap = gt_r[:, :, off:off + n * S].rearrange(
                        "p l (n s) -> p l n s", s=S
                    )
                    out_ap = seg_r[:, grp * GRP:(grp + 1) * GRP, sp:sp + n]
                    nc.vector.reduce_max(out=out_ap, in_=in_ap, axis=AX.X)

            # ---- phase 2: full max, clamp, prefix circles ----
            nc.vector.reduce_max(out=vt_r[:, :, 0:1], in_=seg_r, axis=AX.X)
            nc.scalar.activation(
                out=vt_r[:, :, 1:78], in_=seg_r[:, :, 0:77], func=AF.Relu
            )
            work = bpool.tile([128, _NT * 80], f32, tag="work")
            nc.gpsimd.ap_gather(
                out_ap=work[:], in_ap=vt[:], idxs_ap=idx2_t[:],
                channels=128, num_elems=_NT * _NSLOT, d=1, num_idxs=_NT * 80,
            )
            work_r = work[:].rearrange("p (i q r) -> p i q r", i=_NT, q=4)
            for r in range(1, _MAXR):
                nc.vector.tensor_tensor(
                    out=work_r[:, :, :, r:r + 1],
                    in0=work_r[:, :, :, r - 1:r],
                    in1=work_r[:, :, :, r:r + 1],
                    op=mybir.AluOpType.max,
                )
            nc.scalar.copy(
                out=vt_r[:, :, 78:154].rearrange("p i (q r) -> p i q r", q=4),
                in_=work_r[:, :, :, 1:20],
            )

            # ---- norms: sum over channels of vt^2, per (b, slot) ----
            sq = bpool.tile([128, _NT * _NSLOT], f32, tag="sq")
            nc.scalar.activation(out=sq[:], in_=vt[:], func=AF.Square)
            sq_r = sq[:].rearrange("p (i k) -> p i k", i=_NT)
            nrm = spool.tile([1, _BL * _NSLOT], f32, tag="nrm")
            for b in range(_BL):
                ps = ppool_n.tile([1, _NSLOT], f32, tag="psn")
                for ct in range(_CT):
                    nc.tensor.matmul(
                        ps[:], ones128_t[:], sq_r[:, b * _CT + ct, :],
                        start=(ct == 0), stop=(ct == _CT - 1),
                    )
                nc.scalar.activation(
                    out=nrm[0:1, b * _NSLOT:(b + 1) * _NSLOT], in_=ps[:],
                    func=AF.Sqrt,
                )
            inv = spool.tile([1, _BL * _NSLOT], f32, tag="inv")
            nc.vector.tensor_scalar_add(out=nrm[0:1, :], in0=nrm[0:1, :],
                                        scalar1=_EPS)
            nc.vector.reciprocal(out=inv[0:1, :], in_=nrm[0:1, :])

            # ---- broadcast inv over partitions, multiply, reduce ----
            prod = bpool.tile([128, _NT * _NSLOT], f32, tag="prod")
            prod_r = prod[:].rearrange("p (i k) -> p i k", i=_NT)
            HB = _BL // 2                   # batches per half
            HN = HB * _NSLOT               # 308
            for h in range(2):
                pb = ppool_b.tile([128, HN], f32, tag="pbc")
                nc.tensor.matmul(
                    pb[:], ones1_t[:], inv[0:1, h * HN:(h + 1) * HN],
                    start=True, stop=True,
                )
                pb_b = (
                    pb[:].rearrange("p (b t) -> p b t", b=HB)
                    .unsqueeze(2)
                    .broadcast_to((128, HB, _CT, _NSLOT))
                )
                half = vt_r[:, h * HB * _CT:(h + 1) * HB * _CT, :].rearrange(
                    "p (b c) t -> p b c t", b=HB
                )
                out_half = prod_r[:, h * HB * _CT:(h + 1) * HB * _CT, :].rearrange(
                    "p (b c) t -> p b c t", b=HB
                )
                nc.vector.tensor_tensor(
                    out=out_half, in0=half, in1=pb_b, op=mybir.AluOpType.mult
                )
            outv = spool.tile([128, _NT], f32, tag="outv")
            nc.vector.reduce_sum(out=outv[:], in_=prod_r, axis=AX.X)
            nc.sync.dma_start(out=out_d[:], in_=outv[:])

    nc.finalize()
    return nc


def _get_nc():
    global _NC_CACHE
    if _NC_CACHE is None:
        _NC_CACHE = _build_nc()
    return _NC_CACHE


# --------------------------------------------------------------------------
# host entry point
# --------------------------------------------------------------------------

def _run(x, trace=False):
    from concourse.bass_utils import run_bass_kernel_spmd

    nc = _get_nc()
    t = _TABLES
    x = np.ascontiguousarray(np.asarray(x, dtype=np.float32))
    xs = x.reshape(_NCORES, _BL, _C, _S)
    ones128 = np.ones((128, 1), np.float32)
    ones1 = np.ones((1, 128), np.float32)
    in_maps = [
        {
            "xs": np.ascontiguousarray(xs[c]),
            "idx1": t["idx1_w"],
            "idx2": t["idx2_w"],
            "ones128": ones128,
            "ones1": ones1,
        }
        for c in range(_NCORES)
    ]
    res = run_bass_kernel_spmd(
        nc, in_maps, core_ids=list(range(_NCORES)), trace=trace
    )
    out = np.empty((_B, _C), np.float32)
    for c in range(_NCORES):
        r = np.asarray(res.results[c]["out"])          # [128, 32]
        rr = r.reshape(128, _BL, _CT)                   # [p, b, ct]
        out[c * _BL:(c + 1) * _BL] = rr.transpose(1, 2, 0).reshape(_BL, _C)
    return out.reshape(_B, _C, 1, 1), res


def kernel(x):
    out, _ = _run(x, trace=False)
    return out


# revision 4
# speedup vs baseline: 6.2295x; 6.2295x over previous
"""Trainium2 Bass kernel for nn_CAC_42511586296007 (circular-mask max-pool descriptor).

Reference computation (per batch b, channel c):
  v = l2norm_over_c(max_hw(x)) + sum over 153 circular masks m of
      l2norm_over_c(max_hw(x * m))
Masks: center point + per-quadrant rings/circles of integer radius on 28x28.

Decomposition (verified bit-exact vs the reference mask construction):
  - 77 segment maxes (center + ring(q, r), q in 0..3, r in 1..19) + an 'outer'
    segment (r2 > 361, feeds only the full-map max).
  - circle(q, r) = max(center, ring(q, 1..r)) -- prefix max over r.
  - masked outputs clamp at 0; full max = max over all segments.

Mapping: batch is sharded 8 ways (4 per core). Per core, per batch b:
  1. DMA the 8 channel-tiles [128c x 784] of batch b.
  2. Mirror all 4 quadrants onto one 15x15 geometry with 4 strided ScalarE
     copies (negative strides) into M[c, cell(225), q(4), ct(8)]; invalid
     edge slots get -3e38. Now every ring is one fixed cell-set and tiles/
     quadrants sit in the contiguous d-dimension.
  3. GPSIMD ap_gather with d=32 pulls ring-sorted cell blocks (240 indices
     instead of 976*8 -- ap_gather costs ~27ns/index + ~1.75ns/element).
  4. One DVE reduce_max per radius bucket -> seg[c, t, slot] (zero padding).
  5. Prefix max for circles, relu clamp, channel norms via PE matmuls,
     1/(norm+eps), broadcast, multiply-accumulate over the 154 slots.
"""

import numpy as np

_B, _C, _HH, _WW = 32, 1024, 28, 28
_S = _HH * _WW            # 784
_NCORES = 8
_BL = _B // _NCORES       # 4 batches per core
_CT = _C // 128           # 8 channel tiles
_NT = _BL * _CT           # 32 tiles per core
_MAXR = 20
_NSLOT = 154              # full + center + 76 rings + 76 circles
_NSEG = 84                # 76 rings + 4 center + 4 outer
_EPS = 1e-6
_NEG = -3.0e38
_QUADS = [(1, 1), (-1, 1), (1, -1), (-1, -1)]   # (sign_x, sign_y) ref order


def _build_tables():
    ij = np.arange(15)
    I, J = np.meshgrid(ij, ij, indexing="ij")
    RING = np.ceil(np.sqrt(I * I + J * J)).astype(int)

    buckets = []                                   # (cells, segslot base)
    for r in range(1, _MAXR):
        cells = [(i, j) for i in range(15) for j in range(15) if RING[i, j] == r]
        buckets.append((cells, (r - 1) * 4))
    buckets.append(([(0, 0)], 76))                 # center
    cells_out = [(i, j) for i in range(15) for j in range(15) if RING[i, j] >= _MAXR]
    buckets.append((cells_out, 80))                # outer (full-max only)

    # three gather chunks of <= 80 indices each (pad with idx 0, unreduced)
    chunk_of = lambda k: 0 if k < 9 else (1 if k < 13 else 2)
    chunks = [[], [], []]
    meta = []                                      # (chunk, cell_off, cnt, segbase)
    for k, (cells, segbase) in enumerate(buckets):
        ci = chunk_of(k)
        meta.append((ci, len(chunks[ci]), len(cells), segbase))
        chunks[ci].extend(i * 15 + j for (i, j) in cells)
    idx = []
    for ch in chunks:
        assert len(ch) <= 80
        idx.extend(ch + [0] * (80 - len(ch)))
    idx = np.asarray(idx, dtype=np.int16)          # [240]
    w = idx.reshape(15, 16).T
    idx_w = np.ascontiguousarray(np.tile(w, (8, 1)))   # [128, 15]
    return meta, idx_w


_META, _IDXW = _build_tables()
_NC_CACHE = None


def _build_nc():
    import concourse.bacc as bacc
    import concourse.mybir as mybir
    from concourse.tile import TileContext

    f32 = mybir.dt.float32
    i16 = mybir.dt.int16
    AX = mybir.AxisListType
    AF = mybir.ActivationFunctionType
    MAX = mybir.AluOpType.max
    MULT = mybir.AluOpType.mult

    nc = bacc.Bacc("TRN2")
    xs = nc.dram_tensor("xs", [_BL, _C, _S], f32, kind="ExternalInput")
    idx_d = nc.dram_tensor("idxg", [128, 15], i16, kind="ExternalInput")
    ones128_d = nc.dram_tensor("ones128", [128, 1], f32, kind="ExternalInput")
    ones1_d = nc.dram_tensor("ones1", [1, 128], f32, kind="ExternalInput")
    out_d = nc.dram_tensor("out", [128, _NT], f32, kind="ExternalOutput")

    with TileContext(nc) as tc:
        with (
            tc.tile_pool(name="const", bufs=1) as cpool,
            tc.tile_pool(name="x", bufs=2) as xpool,
            tc.tile_pool(name="m", bufs=2) as mpool,
            tc.tile_pool(name="g", bufs=3) as gpool,
            tc.tile_pool(name="big", bufs=1) as bpool,
            tc.tile_pool(name="sp", bufs=2) as sppool,
            tc.tile_pool(name="small", bufs=2) as smpool,
            tc.tile_pool(name="psn", bufs=2, space="PSUM") as ppool_n,
            tc.tile_pool(name="pbc", bufs=2, space="PSUM") as ppool_b,
        ):
            idx_t = cpool.tile([128, 15], i16, tag="idx")
            nc.sync.dma_start(out=idx_t[:], in_=idx_d[:])
            ones128_t = cpool.tile([128, 1], f32, tag="o128")
            nc.sync.dma_start(out=ones128_t[:], in_=ones128_d[:])
            ones1_t = cpool.tile([1, 128], f32, tag="o1")
            nc.sync.dma_start(out=ones1_t[:], in_=ones1_d[:])

            seg = bpool.tile([128, _NT * _NSEG], f32, tag="seg")
            seg_r = seg[:].rearrange("p (t k) -> p t k", t=_NT)
            vt = bpool.tile([128, _NT * _NSLOT], f32, tag="vt")
            vt_r = vt[:].rearrange("p (t k) -> p t k", t=_NT)
            outv = bpool.tile([128, _NT], f32, tag="outv")

            for b in range(_BL):
                # -- load the 8 channel tiles of batch b --
                xq = xpool.tile([128, _CT * _S], f32, tag="xq")
                for ct in range(_CT):
                    nc.sync.dma_start(
                        out=xq[:, ct * _S:(ct + 1) * _S],
                        in_=xs[b, ct * 128:(ct + 1) * 128, :],
                    )
                xq_v = xq[:].rearrange("p (t a c) -> p t a c", t=_CT, a=_HH)

                # -- mirror quadrants into M[c, cell, q, ct] --
                M = mpool.tile([128, 225 * 32], f32, tag="M")
                M_v = M[:].rearrange(
                    "p (i j q t) -> p i j q t", i=15, j=15, q=4
                )
                # invalid edge slots: (i=14, q in {0,1}), (j=14, q in {0,2})
                nc.gpsimd.memset(M_v[:, 14, :, 0:2, :], _NEG)
                nc.gpsimd.memset(M_v[:, :, 14, 0, :], _NEG)
                nc.gpsimd.memset(M_v[:, :, 14, 2, :], _NEG)
                for qi, (sx, sy) in enumerate(_QUADS):
                    ic = 14 if sy == 1 else 15
                    jc = 14 if sx == 1 else 15
                    src = xq_v[
                        :, :,
                        (slice(14, 14 + ic) if sy == 1 else slice(14, None, -1)),
                        (slice(14, 14 + jc) if sx == 1 else slice(14, None, -1)),
                    ]                                    # [p, t, i, j]
                    nc.scalar.copy(
                        out=M_v[:, 0:ic, 0:jc, qi, :],
                        in_=src.transpose([0, 2, 3, 1]),
                    )

                # -- gather ring-sorted blocks, d=32 --
                gts = []
                for ci in range(3):
                    g = gpool.tile([128, 80 * 32], f32, tag="g")
                    nc.gpsimd.ap_gather(
                        out_ap=g[:], in_ap=M[:],
                        idxs_ap=idx_t[:, ci * 5:(ci + 1) * 5],
                        channels=128, num_elems=225, d=32, num_idxs=80,
                    )
                    gts.append(g)

                # -- one reduce per radius bucket --
                for ci, off, cnt, segbase in _META:
                    blk = gts[ci][:, off * 32:(off + cnt) * 32].rearrange(
                        "p (s q t) -> p t q s", q=4, t=_CT
                    )
                    nc.vector.reduce_max(
                        out=seg_r[:, b * _CT:(b + 1) * _CT, segbase:segbase + 4],
                        in_=blk, axis=AX.X,
                    )

            # -- phase 2, per half (2 batches) --
            for h in range(2):
                ts = slice(h * 16, (h + 1) * 16)
                nc.vector.reduce_max(
                    out=vt_r[:, ts, 0:1], in_=seg_r[:, ts, :], axis=AX.X
                )
                nc.scalar.activation(
                    out=vt_r[:, ts, 1:78], in_=seg_r[:, ts, 0:77], func=AF.Relu
                )
                for r in range(1, _MAXR):
                    in0 = (
                        vt_r[:, ts, 77:78].broadcast_to((128, 16, 4))
                        if r == 1
                        else vt_r[:, ts, 78 + (r - 2) * 4: 78 + (r - 1) * 4]
                    )
                    nc.vector.tensor_tensor(
                        out=vt_r[:, ts, 78 + (r - 1) * 4: 78 + r * 4],
                        in0=in0,
                        in1=vt_r[:, ts, 1 + (r - 1) * 4: 1 + r * 4],
                        op=MAX,
                    )
                for b in (h * 2, h * 2 + 1):
                    bs = slice(b * _CT, (b + 1) * _CT)
                    sq = sppool.tile([128, _CT * _NSLOT], f32, tag="sp")
                    nc.scalar.activation(
                        out=sq[:], in_=vt[:, b * _CT * _NSLOT:(b + 1) * _CT * _NSLOT],
                        func=AF.Square,
                    )
                    sq_v = sq[:].rearrange("p (t k) -> p t k", t=_CT)
                    ps = ppool_n.tile([1, _NSLOT], f32, tag="psn")
                    for ct in range(_CT):
                        nc.tensor.matmul(
                            ps[:], ones128_t[:], sq_v[:, ct, :],
                            start=(ct == 0), stop=(ct == _CT - 1),
                        )
                    nrm = smpool.tile([1, _NSLOT], f32, tag="nrm")
                    nc.scalar.activation(out=nrm[:], in_=ps[:], func=AF.Sqrt)
                    nc.vector.tensor_scalar_add(out=nrm[:], in0=nrm[:], scalar1=_EPS)
                    inv = smpool.tile([1, _NSLOT], f32, tag="inv")
                    scr = smpool.tile([1, _NSLOT], f32, tag="scr")
                    nc.vector.reciprocal_approx_accurate(
                        out=inv[:], in_=nrm[:], scratch=scr[:]
                    )
                    pb = ppool_b.tile([128, _NSLOT], f32, tag="pbc")
                    nc.tensor.matmul(pb[:], ones1_t[:], inv[:], start=True, stop=True)
                    prod = sppool.tile([128, _CT * _NSLOT], f32, tag="sp")
                    prod_v = prod[:].rearrange("p (t k) -> p t k", t=_CT)
                    nc.vector.tensor_tensor(
                        out=prod_v,
                        in0=vt_r[:, bs, :],
                        in1=pb[:].unsqueeze(1).broadcast_to((128, _CT, _NSLOT)),
                        op=MULT,
                    )
                    nc.vector.reduce_sum(
                        out=outv[:, bs], in_=prod_v, axis=AX.X
                    )
            nc.sync.dma_start(out=out_d[:], in_=outv[:])

    nc.finalize()
    return nc


def _get_nc():
    global _NC_CACHE
    if _NC_CACHE is None:
        _NC_CACHE = _build_nc()
    return _NC_CACHE


def _run(x, trace=False):
    from concourse.bass_utils import run_bass_kernel_spmd

    nc = _get_nc()
    x = np.ascontiguousarray(np.asarray(x, dtype=np.float32))
    xs = x.reshape(_NCORES, _BL, _C, _S)
    ones128 = np.ones((128, 1), np.float32)
    ones1 = np.ones((1, 128), np.float32)
    in_maps = [
        {
            "xs": np.ascontiguousarray(xs[c]),
            "idxg": _IDXW,
            "ones128": ones128,
            "ones1": ones1,
        }
        for c in range(_NCORES)
    ]
    res = run_bass_kernel_spmd(
        nc, in_maps, core_ids=list(range(_NCORES)), trace=trace
    )
    out = np.empty((_B, _C), np.float32)
    for c in range(_NCORES):
        r = np.asarray(res.results[c]["out"])           # [128, 32]
        rr = r.reshape(128, _BL, _CT)                    # [p, b, ct]
        out[c * _BL:(c + 1) * _BL] = rr.transpose(1, 2, 0).reshape(_BL, _C)
    return out.reshape(_B, _C, 1, 1), res


def kernel(x):
    out, _ = _run(x, trace=False)
    return out


# revision 5
# speedup vs baseline: 6.2983x; 1.0110x over previous
"""Trainium2 Bass kernel for nn_CAC_42511586296007 (circular-mask max-pool descriptor).

Reference computation (per batch b, channel c):
  v = l2norm_over_c(max_hw(x)) + sum over 153 circular masks m of
      l2norm_over_c(max_hw(x * m))
Masks: center point + per-quadrant rings/circles of integer radius on 28x28.

Decomposition (verified bit-exact vs the reference mask construction):
  - 77 segment maxes (center + ring(q, r), q in 0..3, r in 1..19) + an 'outer'
    segment (r2 > 361, feeds only the full-map max).
  - circle(q, r) = max(center, ring(q, 1..r)) -- prefix max over r.
  - masked outputs clamp at 0; full max = max over all segments.

Mapping: batch is sharded 8 ways (4 per core). Per core, per batch b:
  1. DMA the 8 channel-tiles [128c x 784] of batch b.
  2. Mirror all 4 quadrants onto one 15x15 geometry with 4 strided ScalarE
     copies (negative strides) into M[c, cell(225), q(4), ct(8)]; invalid
     edge slots get -3e38. Now every ring is one fixed cell-set and tiles/
     quadrants sit in the contiguous d-dimension.
  3. GPSIMD ap_gather with d=32 pulls ring-sorted cell blocks (240 indices
     instead of 976*8 -- ap_gather costs ~27ns/index + ~1.75ns/element).
  4. One DVE reduce_max per radius bucket -> seg[c, t, slot] (zero padding).
  5. Prefix max for circles, relu clamp, channel norms via PE matmuls,
     1/(norm+eps), broadcast, multiply-accumulate over the 154 slots.
"""

import numpy as np

_B, _C, _HH, _WW = 32, 1024, 28, 28
_S = _HH * _WW            # 784
_NCORES = 8
_BL = _B // _NCORES       # 4 batches per core
_CT = _C // 128           # 8 channel tiles
_NT = _BL * _CT           # 32 tiles per core
_MAXR = 20
_NSLOT = 154              # full + center + 76 rings + 76 circles
_NSEG = 84                # 76 rings + 4 center + 4 outer
_EPS = 1e-6
_NEG = -3.0e38
_QUADS = [(1, 1), (-1, 1), (1, -1), (-1, -1)]   # (sign_x, sign_y) ref order


def _build_tables():
    ij = np.arange(15)
    I, J = np.meshgrid(ij, ij, indexing="ij")
    RING = np.ceil(np.sqrt(I * I + J * J)).astype(int)

    buckets = []                                   # (cells, segslot base)
    for r in range(1, _MAXR):
        cells = [(i, j) for i in range(15) for j in range(15) if RING[i, j] == r]
        buckets.append((cells, (r - 1) * 4))
    buckets.append(([(0, 0)], 76))                 # center
    cells_out = [(i, j) for i in range(15) for j in range(15) if RING[i, j] >= _MAXR]
    buckets.append((cells_out, 80))                # outer (full-max only)

    # three gather chunks of <= 80 indices each (pad with idx 0, unreduced)
    chunk_of = lambda k: 0 if k < 9 else (1 if k < 13 else 2)
    chunks = [[], [], []]
    meta = []                                      # (chunk, cell_off, cnt, segbase)
    for k, (cells, segbase) in enumerate(buckets):
        ci = chunk_of(k)
        meta.append((ci, len(chunks[ci]), len(cells), segbase))
        chunks[ci].extend(i * 15 + j for (i, j) in cells)
    idx = []
    for ch in chunks:
        assert len(ch) <= 80
        idx.extend(ch + [0] * (80 - len(ch)))
    idx_ws = []
    for ci in range(3):
        a = np.asarray(idx[ci * 80:(ci + 1) * 80], dtype=np.int16)
        w = a.reshape(5, 16).T
        idx_ws.append(np.ascontiguousarray(np.tile(w, (8, 1))))  # [128, 5]
    return meta, idx_ws


_META, _IDXW = _build_tables()
_NC_CACHE = None


def _build_nc():
    import concourse.bacc as bacc
    import concourse.mybir as mybir
    from concourse.tile import TileContext

    f32 = mybir.dt.float32
    i16 = mybir.dt.int16
    AX = mybir.AxisListType
    AF = mybir.ActivationFunctionType
    MAX = mybir.AluOpType.max
    MULT = mybir.AluOpType.mult

    nc = bacc.Bacc("TRN2")
    xs = nc.dram_tensor("xs", [_BL, _C, _S], f32, kind="ExternalInput")
    idx_d = [nc.dram_tensor(f"idxg{ci}", [128, 5], i16, kind="ExternalInput")
             for ci in range(3)]
    ones128_d = nc.dram_tensor("ones128", [128, 1], f32, kind="ExternalInput")
    ones1_d = nc.dram_tensor("ones1", [1, 128], f32, kind="ExternalInput")
    out_d = nc.dram_tensor("out", [128, _NT], f32, kind="ExternalOutput")

    with TileContext(nc) as tc:
        with (
            tc.tile_pool(name="const", bufs=1) as cpool,
            tc.tile_pool(name="x", bufs=2) as xpool,
            tc.tile_pool(name="m", bufs=2) as mpool,
            tc.tile_pool(name="g", bufs=3) as gpool,
            tc.tile_pool(name="big", bufs=1) as bpool,
            tc.tile_pool(name="sp", bufs=2) as sppool,
            tc.tile_pool(name="small", bufs=2) as smpool,
            tc.tile_pool(name="psn", bufs=2, space="PSUM") as ppool_n,
            tc.tile_pool(name="pbc", bufs=2, space="PSUM") as ppool_b,
        ):
            idx_t = []
            for ci in range(3):
                it = cpool.tile([128, 8], i16, tag=f"idx{ci}")
                nc.sync.dma_start(out=it[:, 0:5], in_=idx_d[ci][:])
                idx_t.append(it)
            ones128_t = cpool.tile([128, 1], f32, tag="o128")
            nc.sync.dma_start(out=ones128_t[:], in_=ones128_d[:])
            ones1_t = cpool.tile([1, 128], f32, tag="o1")
            nc.sync.dma_start(out=ones1_t[:], in_=ones1_d[:])

            seg = bpool.tile([128, _NT * _NSEG], f32, tag="seg")
            seg_r = seg[:].rearrange("p (t k) -> p t k", t=_NT)
            vt = bpool.tile([128, _NT * _NSLOT], f32, tag="vt")
            vt_r = vt[:].rearrange("p (t k) -> p t k", t=_NT)
            outv = bpool.tile([128, _NT], f32, tag="outv")

            for b in range(_BL):
                # -- load the 8 channel tiles of batch b --
                xq = xpool.tile([128, _CT * _S], f32, tag="xq")
                for ct in range(_CT):
                    nc.sync.dma_start(
                        out=xq[:, ct * _S:(ct + 1) * _S],
                        in_=xs[b, ct * 128:(ct + 1) * 128, :],
                    )
                xq_v = xq[:].rearrange("p (t a c) -> p t a c", t=_CT, a=_HH)

                # -- mirror quadrants into M[c, cell, q, ct] --
                M = mpool.tile([128, 225 * 32], f32, tag="M")
                M_v = M[:].rearrange(
                    "p (i j q t) -> p i j q t", i=15, j=15, q=4
                )
                # invalid edge slots: (i=14, q in {0,1}), (j=14, q in {0,2})
                nc.gpsimd.memset(M_v[:, 14, :, 0:2, :], _NEG)
                nc.gpsimd.memset(M_v[:, :, 14, 0, :], _NEG)
                nc.gpsimd.memset(M_v[:, :, 14, 2, :], _NEG)
                for qi, (sx, sy) in enumerate(_QUADS):
                    ic = 14 if sy == 1 else 15
                    jc = 14 if sx == 1 else 15
                    src = xq_v[
                        :, :,
                        (slice(14, 14 + ic) if sy == 1 else slice(14, None, -1)),
                        (slice(14, 14 + jc) if sx == 1 else slice(14, None, -1)),
                    ]                                    # [p, t, i, j]
                    nc.scalar.copy(
                        out=M_v[:, 0:ic, 0:jc, qi, :],
                        in_=src.transpose([0, 2, 3, 1]),
                    )

                # -- gather ring-sorted blocks, d=32 --
                gts = []
                for ci in range(3):
                    g = gpool.tile([128, 80 * 32], f32, tag="g")
                    nc.gpsimd.ap_gather(
                        out_ap=g[:], in_ap=M[:],
                        idxs_ap=idx_t[ci][:, 0:5],
                        channels=128, num_elems=225, d=32, num_idxs=80,
                    )
                    gts.append(g)

                # -- one reduce per radius bucket --
                for ci, off, cnt, segbase in _META:
                    blk = gts[ci][:, off * 32:(off + cnt) * 32].rearrange(
                        "p (s q t) -> p t q s", q=4, t=_CT
                    )
                    nc.vector.reduce_max(
                        out=seg_r[:, b * _CT:(b + 1) * _CT, segbase:segbase + 4],
                        in_=blk, axis=AX.X,
                    )

            # -- phase 2, per half (2 batches) --
            for h in range(2):
                ts = slice(h * 16, (h + 1) * 16)
                nc.vector.reduce_max(
                    out=vt_r[:, ts, 0:1], in_=seg_r[:, ts, :], axis=AX.X
                )
                nc.scalar.activation(
                    out=vt_r[:, ts, 1:78], in_=seg_r[:, ts, 0:77], func=AF.Relu
                )
                for r in range(1, _MAXR):
                    in0 = (
                        vt_r[:, ts, 77:78].broadcast_to((128, 16, 4))
                        if r == 1
                        else vt_r[:, ts, 78 + (r - 2) * 4: 78 + (r - 1) * 4]
                    )
                    nc.vector.tensor_tensor(
                        out=vt_r[:, ts, 78 + (r - 1) * 4: 78 + r * 4],
                        in0=in0,
                        in1=vt_r[:, ts, 1 + (r - 1) * 4: 1 + r * 4],
                        op=MAX,
                    )
                for b in (h * 2, h * 2 + 1):
                    bs = slice(b * _CT, (b + 1) * _CT)
                    sq = sppool.tile([128, _CT * _NSLOT], f32, tag="sp")
                    nc.scalar.activation(
                        out=sq[:], in_=vt[:, b * _CT * _NSLOT:(b + 1) * _CT * _NSLOT],
                        func=AF.Square,
                    )
                    sq_v = sq[:].rearrange("p (t k) -> p t k", t=_CT)
                    ps = ppool_n.tile([1, _NSLOT], f32, tag="psn")
                    for ct in range(_CT):
                        nc.tensor.matmul(
                            ps[:], ones128_t[:], sq_v[:, ct, :],
                            start=(ct == 0), stop=(ct == _CT - 1),
                        )
                    nrm = smpool.tile([1, _NSLOT], f32, tag="nrm")
                    nc.scalar.activation(out=nrm[:], in_=ps[:], func=AF.Sqrt)
                    nc.vector.tensor_scalar_add(out=nrm[:], in0=nrm[:], scalar1=_EPS)
                    inv = smpool.tile([1, _NSLOT], f32, tag="inv")
                    scr = smpool.tile([1, _NSLOT], f32, tag="scr")
                    nc.vector.reciprocal_approx_accurate(
                        out=inv[:], in_=nrm[:], scratch=scr[:]
                    )
                    pb = ppool_b.tile([128, _NSLOT], f32, tag="pbc")
                    nc.tensor.matmul(pb[:], ones1_t[:], inv[:], start=True, stop=True)
                    prod = sppool.tile([128, _CT * _NSLOT], f32, tag="sp")
                    prod_v = prod[:].rearrange("p (t k) -> p t k", t=_CT)
                    nc.vector.tensor_tensor(
                        out=prod_v,
                        in0=vt_r[:, bs, :],
                        in1=pb[:].unsqueeze(1).broadcast_to((128, _CT, _NSLOT)),
                        op=MULT,
                    )
                    nc.vector.reduce_sum(
                        out=outv[:, bs], in_=prod_v, axis=AX.X
                    )
            nc.sync.dma_start(out=out_d[:], in_=outv[:])

    nc.finalize()
    return nc


def _get_nc():
    global _NC_CACHE
    if _NC_CACHE is None:
        _NC_CACHE = _build_nc()
    return _NC_CACHE


def _run(x, trace=False):
    from concourse.bass_utils import run_bass_kernel_spmd

    nc = _get_nc()
    x = np.ascontiguousarray(np.asarray(x, dtype=np.float32))
    xs = x.reshape(_NCORES, _BL, _C, _S)
    ones128 = np.ones((128, 1), np.float32)
    ones1 = np.ones((1, 128), np.float32)
    in_maps = [
        {
            "xs": np.ascontiguousarray(xs[c]),
            "idxg0": _IDXW[0], "idxg1": _IDXW[1], "idxg2": _IDXW[2],
            "ones128": ones128,
            "ones1": ones1,
        }
        for c in range(_NCORES)
    ]
    res = run_bass_kernel_spmd(
        nc, in_maps, core_ids=list(range(_NCORES)), trace=trace
    )
    out = np.empty((_B, _C), np.float32)
    for c in range(_NCORES):
        r = np.asarray(res.results[c]["out"])           # [128, 32]
        rr = r.reshape(128, _BL, _CT)                    # [p, b, ct]
        out[c * _BL:(c + 1) * _BL] = rr.transpose(1, 2, 0).reshape(_BL, _C)
    return out.reshape(_B, _C, 1, 1), res


def kernel(x):
    out, _ = _run(x, trace=False)
    return out


# revision 6
# speedup vs baseline: 6.7799x; 1.0765x over previous
"""Trainium2 Bass kernel for nn_CAC_42511586296007 (circular-mask max-pool descriptor).

Reference computation (per batch b, channel c):
  v = l2norm_over_c(max_hw(x)) + sum over 153 circular masks m of
      l2norm_over_c(max_hw(x * m))
Masks: center point + per-quadrant rings/circles of integer radius on 28x28.

Decomposition (verified bit-exact vs the reference mask construction):
  - 77 segment maxes (center + ring(q, r), q in 0..3, r in 1..19) + an 'outer'
    segment (r2 > 361, feeds only the full-map max).
  - circle(q, r) = max(center, ring(q, 1..r)) -- prefix max over r.
  - masked outputs clamp at 0; full max = max over all segments.

Mapping: batch is sharded 8 ways (4 per core). Per core, per batch b:
  1. DMA the 8 channel-tiles [128c x 784] of batch b.
  2. Mirror all 4 quadrants onto one 15x15 geometry with 4 strided ScalarE
     copies (negative strides) into M[c, cell(225), q(4), ct(8)]; invalid
     edge slots get -3e38. Now every ring is one fixed cell-set and tiles/
     quadrants sit in the contiguous d-dimension.
  3. GPSIMD ap_gather with d=32 pulls ring-sorted cell blocks (240 indices
     instead of 976*8 -- ap_gather costs ~27ns/index + ~1.75ns/element).
  4. One DVE reduce_max per radius bucket -> seg[c, t, slot] (zero padding).
  5. Prefix max for circles, relu clamp, channel norms via PE matmuls,
     1/(norm+eps), broadcast, multiply-accumulate over the 154 slots.
"""

import numpy as np

_B, _C, _HH, _WW = 32, 1024, 28, 28
_S = _HH * _WW            # 784
_NCORES = 8
_BL = _B // _NCORES       # 4 batches per core
_CT = _C // 128           # 8 channel tiles
_NT = _BL * _CT           # 32 tiles per core
_MAXR = 20
_NSLOT = 154              # full + center + 76 rings + 76 circles
_NSEG = 84                # 76 rings + 4 center + 4 outer
_EPS = 1e-6
_NEG = -3.0e38
_QUADS = [(1, 1), (-1, 1), (1, -1), (-1, -1)]   # (sign_x, sign_y) ref order


def _build_tables():
    ij = np.arange(15)
    I, J = np.meshgrid(ij, ij, indexing="ij")
    RING = np.ceil(np.sqrt(I * I + J * J)).astype(int)

    buckets = []                                   # (cells, segslot base)
    for r in range(1, _MAXR):
        cells = [(i, j) for i in range(15) for j in range(15) if RING[i, j] == r]
        buckets.append((cells, (r - 1) * 4))
    buckets.append(([(0, 0)], 76))                 # center
    cells_out = [(i, j) for i in range(15) for j in range(15) if RING[i, j] >= _MAXR]
    buckets.append((cells_out, 80))                # outer (full-max only)

    # two gather chunks (pad with idx 0, unreduced): sizes 112 + 128
    CHUNK_PAD = (112, 128)
    chunk_of = lambda k: 0 if k < 11 else 1
    chunks = [[], []]
    meta = []                                      # (chunk, cell_off, cnt, segbase)
    for k, (cells, segbase) in enumerate(buckets):
        ci = chunk_of(k)
        meta.append((ci, len(chunks[ci]), len(cells), segbase))
        chunks[ci].extend(i * 15 + j for (i, j) in cells)
    idx_ws = []
    for ci, ch in enumerate(chunks):
        n = CHUNK_PAD[ci]
        assert len(ch) <= n
        a = np.asarray(ch + [0] * (n - len(ch)), dtype=np.int16)
        w = a.reshape(n // 16, 16).T
        idx_ws.append(np.ascontiguousarray(np.tile(w, (8, 1))))  # [128, n//16]
    return meta, idx_ws


_META, _IDXW = _build_tables()
_NC_CACHE = None


def _build_nc():
    import concourse.bacc as bacc
    import concourse.mybir as mybir
    from concourse.tile import TileContext

    f32 = mybir.dt.float32
    i16 = mybir.dt.int16
    AX = mybir.AxisListType
    AF = mybir.ActivationFunctionType
    MAX = mybir.AluOpType.max
    MULT = mybir.AluOpType.mult

    nc = bacc.Bacc("TRN2")
    xs = nc.dram_tensor("xs", [_BL, _C, _S], f32, kind="ExternalInput")
    idx_d = [nc.dram_tensor(f"idxg{ci}", [128, _IDXW[ci].shape[1]], i16,
                            kind="ExternalInput") for ci in range(2)]
    ones128_d = nc.dram_tensor("ones128", [128, 1], f32, kind="ExternalInput")
    ones1_d = nc.dram_tensor("ones1", [1, 128], f32, kind="ExternalInput")
    out_d = nc.dram_tensor("out", [128, _NT], f32, kind="ExternalOutput")

    with TileContext(nc) as tc:
        with (
            tc.tile_pool(name="const", bufs=1) as cpool,
            tc.tile_pool(name="x", bufs=2) as xpool,
            tc.tile_pool(name="m", bufs=2) as mpool,
            tc.tile_pool(name="g", bufs=2) as gpool,
            tc.tile_pool(name="big", bufs=1) as bpool,
            tc.tile_pool(name="sp", bufs=2) as sppool,
            tc.tile_pool(name="small", bufs=2) as smpool,
            tc.tile_pool(name="psn", bufs=2, space="PSUM") as ppool_n,
            tc.tile_pool(name="pbc", bufs=2, space="PSUM") as ppool_b,
        ):
            idx_t = []
            for ci in range(2):
                nci = _IDXW[ci].shape[1]
                it = cpool.tile([128, nci], i16, tag=f"idx{ci}")
                nc.sync.dma_start(out=it[:], in_=idx_d[ci][:])
                idx_t.append(it)
            ones128_t = cpool.tile([128, 1], f32, tag="o128")
            nc.sync.dma_start(out=ones128_t[:], in_=ones128_d[:])
            ones1_t = cpool.tile([1, 128], f32, tag="o1")
            nc.sync.dma_start(out=ones1_t[:], in_=ones1_d[:])

            seg = bpool.tile([128, _NT * _NSEG], f32, tag="seg")
            seg_r = seg[:].rearrange("p (t k) -> p t k", t=_NT)
            vt = bpool.tile([128, _NT * _NSLOT], f32, tag="vt")
            vt_r = vt[:].rearrange("p (t k) -> p t k", t=_NT)
            outv = bpool.tile([128, _NT], f32, tag="outv")

            for b in range(_BL):
                # -- load the 8 channel tiles of batch b --
                xq = xpool.tile([128, _CT * _S], f32, tag="xq")
                for ct in range(_CT):
                    nc.sync.dma_start(
                        out=xq[:, ct * _S:(ct + 1) * _S],
                        in_=xs[b, ct * 128:(ct + 1) * 128, :],
                    )
                xq_v = xq[:].rearrange("p (t a c) -> p t a c", t=_CT, a=_HH)

                # -- mirror quadrants into M[c, cell, q, ct] --
                M = mpool.tile([128, 225 * 32], f32, tag="M")
                M_v = M[:].rearrange(
                    "p (i j q t) -> p i j q t", i=15, j=15, q=4
                )
                # invalid edge slots: (i=14, q in {0,1}), (j=14, q in {0,2})
                nc.vector.memset(M_v[:, 14, :, 0:2, :], _NEG)
                nc.vector.memset(M_v[:, :, 14, 0, :], _NEG)
                nc.vector.memset(M_v[:, :, 14, 2, :], _NEG)
                for qi, (sx, sy) in enumerate(_QUADS):
                    ic = 14 if sy == 1 else 15
                    jc = 14 if sx == 1 else 15
                    src = xq_v[
                        :, :,
                        (slice(14, 14 + ic) if sy == 1 else slice(14, None, -1)),
                        (slice(14, 14 + jc) if sx == 1 else slice(14, None, -1)),
                    ]                                    # [p, t, i, j]
                    nc.scalar.copy(
                        out=M_v[:, 0:ic, 0:jc, qi, :],
                        in_=src.transpose([0, 2, 3, 1]),
                    )

                # -- gather ring-sorted blocks, d=32 --
                gts = []
                for ci in range(2):
                    ni = _IDXW[ci].shape[1] * 16
                    g = gpool.tile([128, 128 * 32], f32, tag="g")
                    nc.gpsimd.ap_gather(
                        out_ap=g[:, 0:ni * 32], in_ap=M[:],
                        idxs_ap=idx_t[ci][:],
                        channels=128, num_elems=225, d=32, num_idxs=ni,
                    )
                    gts.append(g)

                # -- one reduce per radius bucket --
                for ci, off, cnt, segbase in _META:
                    blk = gts[ci][:, off * 32:(off + cnt) * 32].rearrange(
                        "p (s q t) -> p t q s", q=4, t=_CT
                    )
                    nc.vector.reduce_max(
                        out=seg_r[:, b * _CT:(b + 1) * _CT, segbase:segbase + 4],
                        in_=blk, axis=AX.X,
                    )

            # -- phase 2 --
            for h in range(2):
                ts = slice(h * 16, (h + 1) * 16)
                nc.vector.reduce_max(
                    out=vt_r[:, ts, 0:1], in_=seg_r[:, ts, :], axis=AX.X
                )
                nc.scalar.activation(
                    out=vt_r[:, ts, 1:78], in_=seg_r[:, ts, 0:77], func=AF.Relu
                )
            for r in range(1, _MAXR):
                in0 = (
                    vt_r[:, :, 77:78].broadcast_to((128, _NT, 4))
                    if r == 1
                    else vt_r[:, :, 78 + (r - 2) * 4: 78 + (r - 1) * 4]
                )
                nc.vector.tensor_tensor(
                    out=vt_r[:, :, 78 + (r - 1) * 4: 78 + r * 4],
                    in0=in0,
                    in1=vt_r[:, :, 1 + (r - 1) * 4: 1 + r * 4],
                    op=MAX,
                )
            for h in range(2):
                for b in (h * 2, h * 2 + 1):
                    bs = slice(b * _CT, (b + 1) * _CT)
                    sq = sppool.tile([128, _CT * _NSLOT], f32, tag="sp")
                    nc.scalar.activation(
                        out=sq[:], in_=vt[:, b * _CT * _NSLOT:(b + 1) * _CT * _NSLOT],
                        func=AF.Square,
                    )
                    sq_v = sq[:].rearrange("p (t k) -> p t k", t=_CT)
                    ps = ppool_n.tile([1, _NSLOT], f32, tag="psn")
                    for ct in range(_CT):
                        nc.tensor.matmul(
                            ps[:], ones128_t[:], sq_v[:, ct, :],
                            start=(ct == 0), stop=(ct == _CT - 1),
                        )
                    nrm = smpool.tile([1, _NSLOT], f32, tag="nrm")
                    nc.scalar.activation(out=nrm[:], in_=ps[:], func=AF.Sqrt)
                    inv = smpool.tile([1, _NSLOT], f32, tag="inv")
                    scr = smpool.tile([1, _NSLOT], f32, tag="scr")
                    nc.vector.reciprocal_approx_accurate(
                        out=inv[:], in_=nrm[:], scratch=scr[:]
                    )
                    pb = ppool_b.tile([128, _NSLOT], f32, tag="pbc")
                    nc.tensor.matmul(pb[:], ones1_t[:], inv[:], start=True, stop=True)
                    prod = sppool.tile([128, _CT * _NSLOT], f32, tag="sp")
                    prod_v = prod[:].rearrange("p (t k) -> p t k", t=_CT)
                    nc.vector.tensor_tensor(
                        out=prod_v,
                        in0=vt_r[:, bs, :],
                        in1=pb[:].unsqueeze(1).broadcast_to((128, _CT, _NSLOT)),
                        op=MULT,
                    )
                    nc.vector.reduce_sum(
                        out=outv[:, bs], in_=prod_v, axis=AX.X
                    )
            nc.sync.dma_start(out=out_d[:], in_=outv[:])

    nc.finalize()
    return nc


def _get_nc():
    global _NC_CACHE
    if _NC_CACHE is None:
        _NC_CACHE = _build_nc()
    return _NC_CACHE


def _run(x, trace=False):
    from concourse.bass_utils import run_bass_kernel_spmd

    nc = _get_nc()
    x = np.ascontiguousarray(np.asarray(x, dtype=np.float32))
    xs = x.reshape(_NCORES, _BL, _C, _S)
    ones128 = np.ones((128, 1), np.float32)
    ones1 = np.ones((1, 128), np.float32)
    in_maps = [
        {
            "xs": np.ascontiguousarray(xs[c]),
            "idxg0": _IDXW[0], "idxg1": _IDXW[1],
            "ones128": ones128,
            "ones1": ones1,
        }
        for c in range(_NCORES)
    ]
    res = run_bass_kernel_spmd(
        nc, in_maps, core_ids=list(range(_NCORES)), trace=trace
    )
    out = np.empty((_B, _C), np.float32)
    for c in range(_NCORES):
        r = np.asarray(res.results[c]["out"])           # [128, 32]
        rr = r.reshape(128, _BL, _CT)                    # [p, b, ct]
        out[c * _BL:(c + 1) * _BL] = rr.transpose(1, 2, 0).reshape(_BL, _C)
    return out.reshape(_B, _C, 1, 1), res


def kernel(x):
    out, _ = _run(x, trace=False)
    return out
